# revision 9
# baseline (speedup 1.0000x reference)
"""DAG-constraint layer kernel for Trainium2 (8 NeuronCores, data parallel).

The reference computes p = sigmoid(x) followed by an iterative min/max
projection over a fixed chain+skip DAG on N=32 nodes (children of i are
{i+1, i+2}).  On that DAG the projection's fixed point is reached after a
single iteration and collapses to the prefix-min along the node axis:

    out[b, j] = min_{k <= j} sigmoid(x[b, k]) = sigmoid(cummin(x, axis=1))

(verified bitwise against the reference).  So the kernel is a per-row
prefix-min over 32 columns plus a sigmoid - purely memory bound.

Per core: rows are sharded 8 ways (65536 rows x 32 f32 = 8 MiB per shard).
The shard is processed as [128 partitions x F free] tiles; each partition
holds F/32 complete rows, so each row's 32 columns are contiguous in the
free dimension.  The prefix-min of many rows is computed with one hardware
scan instruction (TensorTensorScanArith) per tile:

    state_t = max( min(x_t, state_{t-1}), C_t )

where C is a constant: +BIG at each row's LAST column (t % 32 == 31) and
-BIG elsewhere.  The +BIG poisons the state at each row end, so the next
row starts a fresh running min (initial=+BIG handles the first row).  Each
row's column 31 then holds +BIG instead of the true value; one cheap
strided min (64 elements/partition) repairs it:
    q[:, 31::32] = min(q[:, 30::32], x[:, 31::32])
Sigmoid runs on the scalar engine in place.

Raw bass (explicit semaphores) rather than Tile: the walrus build in this
container only encodes a single sync-wait per instruction, so waits are
issued as standalone wait_ge commands.  Pipeline: sync engine issues input
DMAs, vector (DVE) runs scan+fix, scalar (ACT) runs sigmoid and issues
output DMAs.  Both DMA streams are HWDGE and FIFO per issuing engine, so
cumulative semaphore counts are ordered.
"""

from contextlib import ExitStack

import numpy as np

import concourse.bass as bass
import concourse.mybir as mybir
from concourse.bass_utils import run_bass_kernel_spmd

N_CORES = 8
B_TOTAL = 524288
N_NODES = 32
ROWS_PER_CORE = B_TOTAL // N_CORES  # 65536
P = 128                             # SBUF partitions
F = 2048                            # free elements per partition per tile
TILE_ELEMS = P * F                  # 1 MiB of f32 per tile
NT = ROWS_PER_CORE * N_NODES // TILE_ELEMS  # 8 tiles per core
NB = 3                              # pipeline slots
NEG_BIG = -3.0e38
POS_BIG = 3.0e38

assert F % N_NODES == 0 and ROWS_PER_CORE * N_NODES % TILE_ELEMS == 0


def _col(ap, c):
    """AP selecting column c of every N_NODES-wide row: [P, F/N] stride N."""
    return ap[:].rearrange("p (g n) -> p g n", n=N_NODES)[:, :, c]


def _build() -> bass.Bass:
    nc = bass.Bass()
    f32 = mybir.dt.float32
    x = nc.declare_dram_parameter("x", [ROWS_PER_CORE, N_NODES], f32, isOutput=False)
    y = nc.declare_dram_parameter("y", [ROWS_PER_CORE, N_NODES], f32, isOutput=True)
    xt_all = x[:].flatten().rearrange("(t p f) -> t p f", p=P, f=F)
    yt_all = y[:].flatten().rearrange("(t p f) -> t p f", p=P, f=F)

    with ExitStack() as es:
        ec = es.enter_context
        xts = [ec(nc.sbuf_tensor(f"xt{i}", [P, F], f32)) for i in range(NB)]
        qts = [ec(nc.sbuf_tensor(f"qt{i}", [P, F], f32)) for i in range(NB)]
        cmask = ec(nc.sbuf_tensor("cmask", [P, F], f32))
        # Per-slot DMA semaphores: a cumulative count over several in-flight
        # DMAs is NOT a completion indicator (the 16 per-SDMA-engine
        # increments of different DMAs interleave), but with one outstanding
        # DMA per semaphore the count is exact.
        dma_in = [ec(nc.semaphore(f"dma_in{i}")) for i in range(NB)]
        dma_out = [ec(nc.semaphore(f"dma_out{i}")) for i in range(NB)]
        vec_sem = ec(nc.semaphore("vec_sem"))
        act_sem = ec(nc.semaphore("act_sem"))

        with nc.Block() as block:

            @block.sync
            def _(sync):
                for t in range(NT):
                    if t >= NB:
                        # xt slot free once scan+fix of iteration t-NB ran
                        sync.wait_ge(vec_sem, t - NB + 1)
                    sync.dma_start(
                        out=xts[t % NB][:], in_=xt_all[t]
                    ).then_inc(dma_in[t % NB], 16)
                for s in range(NB):
                    sync.wait_ge(dma_in[s], 16 * ((NT - 1 - s) // NB + 1))

            @block.vector
            def _(vector):
                vector.memset(cmask[:], NEG_BIG)
                vector.memset(_col(cmask, N_NODES - 1), POS_BIG)
                for t in range(NT):
                    s = t % NB
                    vector.wait_ge(dma_in[s], 16 * (t // NB + 1))
                    if t >= NB:
                        # qt slot free once output DMA of iteration t-NB landed
                        vector.wait_ge(dma_out[s], 16 * (t // NB))
                    vector.tensor_tensor_scan(
                        out=qts[s][:],
                        data0=xts[s][:],
                        data1=cmask[:],
                        initial=POS_BIG,
                        op0=mybir.AluOpType.min,
                        op1=mybir.AluOpType.max,
                    )
                    vector.tensor_tensor(
                        out=_col(qts[s], N_NODES - 1),
                        in0=_col(qts[s], N_NODES - 2),
                        in1=_col(xts[s], N_NODES - 1),
                        op=mybir.AluOpType.min,
                    ).then_inc(vec_sem, 1)

            @block.scalar
            def _(scalar):
                for t in range(NT):
                    s = t % NB
                    scalar.wait_ge(vec_sem, t + 1)
                    scalar.activation(
                        out=qts[s][:],
                        in_=qts[s][:],
                        func=mybir.ActivationFunctionType.Sigmoid,
                    ).then_inc(act_sem, 1)
                    # The sequencer dispatches the DMA before the ACTIVATE's
                    # writes land; gate on its completion explicitly.
                    scalar.wait_ge(act_sem, t + 1)
                    scalar.dma_start(
                        out=yt_all[t], in_=qts[s][:]
                    ).then_inc(dma_out[s], 16)
                for s in range(NB):
                    scalar.wait_ge(dma_out[s], 16 * ((NT - 1 - s) // NB + 1))

    return nc


def _run(x: np.ndarray, trace: bool = False):
    x = np.ascontiguousarray(np.asarray(x), dtype=np.float32)
    assert x.shape == (B_TOTAL, N_NODES), x.shape
    nc = _build()
    in_maps = [
        {"x": x[i * ROWS_PER_CORE : (i + 1) * ROWS_PER_CORE]} for i in range(N_CORES)
    ]
    res = run_bass_kernel_spmd(nc, in_maps, list(range(N_CORES)), trace=trace)
    out = np.concatenate([res.results[i]["y"] for i in range(N_CORES)], axis=0)
    return out, res


def kernel(x, children=None, child_mask=None, parents=None, parent_mask=None,
           topo=None, **_unused):
    out, _ = _run(x)
    return out


# revision 13
# speedup vs baseline: 1.0957x; 1.0957x over previous
"""DAG-constraint layer kernel for Trainium2 (8 NeuronCores, data parallel).

The reference computes p = sigmoid(x) followed by an iterative min/max
projection over a fixed chain+skip DAG on N=32 nodes (children of i are
{i+1, i+2}).  On that DAG the projection's fixed point is reached after a
single iteration and collapses to the prefix-min along the node axis:

    out[b, j] = min_{k <= j} sigmoid(x[b, k]) = sigmoid(cummin(x, axis=1))

(verified bitwise against the reference).  So the kernel is a per-row
prefix-min over 32 columns plus a sigmoid - purely memory bound.

Per core: rows are sharded 8 ways (65536 rows x 32 f32 = 8 MiB per shard).
The shard is processed as [128 partitions x F free] tiles; each partition
holds F/32 complete rows, so each row's 32 columns are contiguous in the
free dimension.  The prefix-min of many rows is computed with one hardware
scan instruction (TensorTensorScanArith) per tile:

    state_t = max( min(x_t, state_{t-1}), C_t )

where C is a constant: +BIG at each row's LAST column (t % 32 == 31) and
-BIG elsewhere.  The +BIG poisons the state at each row end, so the next
row starts a fresh running min (initial=+BIG handles the first row).  Each
row's column 31 then holds +BIG instead of the true value; one cheap
strided min (64 elements/partition) repairs it:
    q[:, 31::32] = min(q[:, 30::32], x[:, 31::32])
Sigmoid runs on the scalar engine in place.

Raw bass (explicit semaphores) rather than Tile: the walrus build in this
container only encodes a single sync-wait per instruction, so waits are
issued as standalone wait_ge commands.  Pipeline: sync engine issues input
DMAs, vector (DVE) runs scan+fix, scalar (ACT) runs sigmoid and issues
output DMAs.  Both DMA streams are HWDGE and FIFO per issuing engine, so
cumulative semaphore counts are ordered.
"""

from contextlib import ExitStack

import numpy as np

import concourse.bass as bass
import concourse.mybir as mybir
from concourse.bass_utils import run_bass_kernel_spmd

N_CORES = 8
B_TOTAL = 524288
N_NODES = 32
ROWS_PER_CORE = B_TOTAL // N_CORES  # 65536
P = 128                             # SBUF partitions
F = 2048                            # free elements per partition per tile
TILE_ELEMS = P * F                  # 1 MiB of f32 per tile
NT = ROWS_PER_CORE * N_NODES // TILE_ELEMS  # 8 tiles per core
NEG_BIG = -3.0e38
POS_BIG = 3.0e38

assert F % N_NODES == 0 and ROWS_PER_CORE * N_NODES % TILE_ELEMS == 0


def _col(ap, c):
    """AP selecting column c of every N_NODES-wide row: [P, F/N] stride N."""
    return ap[:].rearrange("p (g n) -> p g n", n=N_NODES)[:, :, c]


def _build() -> bass.Bass:
    nc = bass.Bass()
    f32 = mybir.dt.float32
    x = nc.declare_dram_parameter("x", [ROWS_PER_CORE, N_NODES], f32, isOutput=False)
    y = nc.declare_dram_parameter("y", [ROWS_PER_CORE, N_NODES], f32, isOutput=True)
    xt_all = x[:].flatten().rearrange("(t p f) -> t p f", p=P, f=F)
    yt_all = y[:].flatten().rearrange("(t p f) -> t p f", p=P, f=F)

    with ExitStack() as es:
        ec = es.enter_context
        # All NT tiles resident at once (17 MiB of SBUF): no slot reuse, so
        # the input DMA stream runs with no dependency on compute at all.
        xts = [ec(nc.sbuf_tensor(f"xt{i}", [P, F], f32)) for i in range(NT)]
        qts = [ec(nc.sbuf_tensor(f"qt{i}", [P, F], f32)) for i in range(NT)]
        cmask = ec(nc.sbuf_tensor("cmask", [P, F], f32))
        warm = ec(nc.sbuf_tensor("act_warm", [P, 1], f32))
        # Per-tile input semaphores: a cumulative count over several
        # in-flight DMAs is NOT a completion indicator (the 16 per-SDMA-
        # engine increments of different DMAs interleave), but with one DMA
        # per semaphore the count is exact.  The single output semaphore is
        # only ever waited at its total (all increments fired), so a shared
        # counter is fine there.
        dma_in = [ec(nc.semaphore(f"dma_in{i}")) for i in range(NT)]
        dma_out = ec(nc.semaphore("dma_out"))
        vec_sem = ec(nc.semaphore("vec_sem"))
        act_sem = ec(nc.semaphore("act_sem"))

        with nc.Block() as block:

            @block.sync
            def _(sync):
                for t in range(NT):
                    sync.dma_start(
                        out=xts[t][:], in_=xt_all[t]
                    ).then_inc(dma_in[t], 16)

            @block.vector
            def _(vector):
                vector.memset(cmask[:], NEG_BIG)
                vector.memset(_col(cmask, N_NODES - 1), POS_BIG)
                for t in range(NT):
                    vector.wait_ge(dma_in[t], 16)
                    vector.tensor_tensor_scan(
                        out=qts[t][:],
                        data0=xts[t][:],
                        data1=cmask[:],
                        initial=POS_BIG,
                        op0=mybir.AluOpType.min,
                        op1=mybir.AluOpType.max,
                    )
                    vector.tensor_tensor(
                        out=_col(qts[t], N_NODES - 1),
                        in0=_col(qts[t], N_NODES - 2),
                        in1=_col(xts[t], N_NODES - 1),
                        op=mybir.AluOpType.min,
                    ).then_inc(vec_sem, 1)

            @block.scalar
            def _(scalar):
                # Dummy activation: pulls the sigmoid table load (~2.7us)
                # off the first tile's critical path.  Contents are unused,
                # so the uninitialized tile is fine.
                scalar.activation(
                    out=warm[:], in_=warm[:],
                    func=mybir.ActivationFunctionType.Sigmoid,
                )
                for t in range(NT):
                    scalar.wait_ge(vec_sem, t + 1)
                    scalar.activation(
                        out=qts[t][:],
                        in_=qts[t][:],
                        func=mybir.ActivationFunctionType.Sigmoid,
                    ).then_inc(act_sem, 1)
                    # The sequencer dispatches the DMA before the ACTIVATE's
                    # writes land; gate on its completion explicitly.
                    scalar.wait_ge(act_sem, t + 1)
                    scalar.dma_start(
                        out=yt_all[t], in_=qts[t][:]
                    ).then_inc(dma_out, 16)
                scalar.wait_ge(dma_out, 16 * NT)

    return nc


def _run(x: np.ndarray, trace: bool = False):
    x = np.ascontiguousarray(np.asarray(x), dtype=np.float32)
    assert x.shape == (B_TOTAL, N_NODES), x.shape
    nc = _build()
    in_maps = [
        {"x": x[i * ROWS_PER_CORE : (i + 1) * ROWS_PER_CORE]} for i in range(N_CORES)
    ]
    res = run_bass_kernel_spmd(nc, in_maps, list(range(N_CORES)), trace=trace)
    out = np.concatenate([res.results[i]["y"] for i in range(N_CORES)], axis=0)
    return out, res


def kernel(x, children=None, child_mask=None, parents=None, parent_mask=None,
           topo=None, **_unused):
    out, _ = _run(x)
    return out
